# revision 8
# baseline (speedup 1.0000x reference)
"""Trainium2 Bass kernel for nn_DecoderUnit: additive attention + GRUCell +
vocab projection, data-parallel over batch B=256 across 8 NeuronCores.

Per core (B_LOC=32), fp16 matmuls / fp32 accumulate:
  xProj: b-pairs, rhs N=512, lhsT=xe_w.T chunks            (PE)
  tanh(xProj + sProj + xe_b + se_b): per-partition bias     (ACT)
  scores: M=1 matmuls (lhsT=we_w [128,1]) -> [1,512]/pair   (PE -> DVE -> DMA)
  softmax per 8-batch group, software-pipelined emission    (DVE/ACT)
  context: M=1 matmuls (lhsT=alphaT col, rhs=x natural)     (PE -> DVE -> DMA)
  GRU in natural [b, 3SD] layout, biases broadcast          (PE/DVE/ACT)
  logits = h @ fc_w.T + fc_b, fc_w.T streamed               (PE)
"""
import functools
import numpy as np

import concourse.bass as bass
import concourse.mybir as mybir
import concourse.tile as tile
from concourse import bacc
from concourse.bass_utils import run_bass_kernel_spmd
from concourse.masks import make_identity

B, T, XD, SD, AD, YD = 256, 256, 512, 512, 512, 6625
NCORES = 8
BL = B // NCORES          # 32
G = 8                     # softmax group size
NG = BL // G              # 4 groups
GP2 = G // 2              # 4 pairs per group
NYB = 13
YB = 512
N_HEAT = 24

f32 = mybir.dt.float32
f16 = mybir.dt.float16
i32 = mybir.dt.int32
OP = mybir.AluOpType
AF = mybir.ActivationFunctionType
AX = mybir.AxisListType


def _build():
    nc = bacc.Bacc(name="decoder_unit")

    xT_d = nc.dram_tensor("xT", [BL // 2, XD, 2 * T], f16, kind="ExternalInput")
    xn_d = nc.dram_tensor("xn", [BL, T, XD], f16, kind="ExternalInput")
    sT_d = nc.dram_tensor("sT", [SD, BL], f32, kind="ExternalInput")
    sn_d = nc.dram_tensor("sn", [BL, SD], f32, kind="ExternalInput")
    yidx_d = nc.dram_tensor("yidx", [BL, 1], i32, kind="ExternalInput")
    ident_d = nc.dram_tensor("ident", [128, 128], f32, kind="ExternalInput")
    xewT_d = nc.dram_tensor("xewT", [XD, AD], f16, kind="ExternalInput")
    sewT_d = nc.dram_tensor("sewT", [SD, AD], f16, kind="ExternalInput")
    wewT_d = nc.dram_tensor("wewT", [AD, 1], f16, kind="ExternalInput")
    xsb_d = nc.dram_tensor("xsb", [AD, 1], f32, kind="ExternalInput")
    emb_d = nc.dram_tensor("emb", [YD + 1, AD], f32, kind="ExternalInput")
    wihT_d = nc.dram_tensor("wihT", [AD + XD, 3 * SD], f16, kind="ExternalInput")
    whhT_d = nc.dram_tensor("whhT", [SD, 3 * SD], f16, kind="ExternalInput")
    bih_d = nc.dram_tensor("bih", [1, 3 * SD], f32, kind="ExternalInput")
    bhh_d = nc.dram_tensor("bhh", [1, 3 * SD], f32, kind="ExternalInput")
    fcwT_d = nc.dram_tensor("fcwT", [SD, YD], f16, kind="ExternalInput")
    fcb_d = nc.dram_tensor("fcb", [1, YD], f32, kind="ExternalInput")
    logits_d = nc.dram_tensor("logits", [BL, YD], f32, kind="ExternalOutput")
    h_d = nc.dram_tensor("h_out", [BL, SD], f32, kind="ExternalOutput")

    fcwT_ap = fcwT_d[:, :].rearrange("(c p) y -> p c y", p=128)

    with tile.TileContext(nc) as tc:
        with (
            tc.tile_pool(name="const", bufs=1) as C,
            tc.tile_pool(name="sm", bufs=3) as SM,
            tc.tile_pool(name="xt", bufs=4) as XT,
            tc.tile_pool(name="xn", bufs=4) as XN,
            tc.tile_pool(name="th", bufs=3) as TH,
            tc.tile_pool(name="fcw", bufs=10) as FW,
            tc.tile_pool(name="tp_ps", bufs=1, space="PSUM") as TP,
        ):
            # --- earliest DMAs: xe_w.T + first x pair tiles ---
            ident = C.tile([128, 128], f32)
            nc.sync.dma_start(out=ident[:], in_=ident_d[:, :])
            xewT_sb = C.tile([128, 4, AD], f16)
            nc.sync.dma_start(out=xewT_sb[:], in_=xewT_d[:, :].rearrange("(c p) a -> p c a", p=128))
            xt_tiles = {}
            for p in range(2):
                t_ = XT.tile([128, 4, 2 * T], f16, tag="xt_sb")
                nc.sync.dma_start(out=t_[:], in_=xT_d[p, :, :].rearrange("(c p2) t -> p2 c t", p2=128))
                xt_tiles[p] = t_

            # --- setup ---
            sT_sb = C.tile([128, 4, BL], f32)
            nc.sync.dma_start(out=sT_sb[:], in_=sT_d[:, :].rearrange("(c p) b -> p c b", p=128))
            sT16 = C.tile([128, 4, BL], f16)
            nc.vector.tensor_copy(out=sT16[:], in_=sT_sb[:])
            sewT_sb = C.tile([128, 4, AD], f16)
            nc.sync.dma_start(out=sewT_sb[:], in_=sewT_d[:, :].rearrange("(c p) a -> p c a", p=128))
            wew_sb = C.tile([128, 4, 1], f16)
            nc.sync.dma_start(out=wew_sb[:], in_=wewT_d[:, :].rearrange("(c p) o -> p c o", p=128))
            xsb_sb = C.tile([128, 4, 1], f32)
            nc.sync.dma_start(out=xsb_sb[:], in_=xsb_d[:, :].rearrange("(c p) o -> p c o", p=128))

            warm = TP.tile([128, 128], f32, tag="tp")
            nc.tensor.transpose(out=warm[:], in_=ident[:], identity=ident[:])

            sProjB_sb = C.tile([128, 4, BL], f32)
            with tc.tile_pool(name="sproj_ps", bufs=1, space="PSUM") as SPP:
                sp_ps = SPP.tile([128, 4, BL], f32)
                for m in range(4):
                    for k in range(4):
                        nc.tensor.matmul(sp_ps[:, m, :],
                                         lhsT=sewT_sb[:, k, m * 128:(m + 1) * 128],
                                         rhs=sT16[:, k, :],
                                         start=(k == 0), stop=(k == 3))
                for m in range(4):
                    nc.vector.tensor_scalar(out=sProjB_sb[:, m, :], in0=sp_ps[:, m, :],
                                            scalar1=xsb_sb[:, m, :], scalar2=None,
                                            op0=OP.add)

            # --- embedding -> ccT chunks 0..3 (early: no attention deps) ---
            idx_sb = C.tile([BL, 1], i32)
            nc.sync.dma_start(out=idx_sb[:], in_=yidx_d[:, :])
            yproj_sb = C.tile([BL, AD], f32)
            nc.gpsimd.indirect_dma_start(
                out=yproj_sb[:], out_offset=None, in_=emb_d[:, :],
                in_offset=bass.IndirectOffsetOnAxis(ap=idx_sb[:, :1], axis=0))
            yproj2 = C.tile([BL, AD], f32)
            nc.vector.tensor_copy(out=yproj2[:], in_=yproj_sb[:])
            ccT_sb = C.tile([128, 8, BL], f16)
            for c in range(4):
                ydT = TP.tile([128, BL], f32, tag="tp")
                nc.tensor.transpose(out=ydT[:], in_=yproj2[:, c * 128:(c + 1) * 128],
                                    identity=ident[:BL, :BL])
                nc.vector.tensor_copy(out=ccT_sb[:, c, :], in_=ydT[:])

            ctx_nat = C.tile([BL, XD], f32)
            wihT_sb = C.tile([128, 8, 3 * SD], f16)
            whhT_sb = C.tile([128, 4, 3 * SD], f16)
            bih_bc = C.tile([BL, 3 * SD], f32)
            bhh_bc = C.tile([BL, 3 * SD], f32)
            sn_sb = C.tile([BL, SD], f32)

            # ================= attention: software-pipelined groups ========
            with (
                tc.tile_pool(name="xp_ps", bufs=4, space="PSUM") as XPP,
                tc.tile_pool(name="sc_ps", bufs=1, space="PSUM") as SCP,
                tc.tile_pool(name="cx_ps", bufs=2, space="PSUM") as CXP,
            ):
                group_state = {}

                def emit_pair(p, th_prev, pl_ctx=None, g_ctx=None):
                    """xProj + tanh for pair p; scores for previous pair."""
                    if p in xt_tiles:
                        xt_sb = xt_tiles.pop(p)
                    else:
                        xt_sb = XT.tile([128, 4, 2 * T], f16, tag="xt_sb")
                        nc.sync.dma_start(
                            out=xt_sb[:],
                            in_=xT_d[p, :, :].rearrange("(c p2) t -> p2 c t", p2=128))
                    th_sb = TH.tile([128, 4, 2 * T], f16, tag="th_sb")
                    for m in range(4):
                        xp_ps = XPP.tile([128, 2 * T], f32, tag="xp")
                        for k in range(4):
                            nc.tensor.matmul(xp_ps[:],
                                             lhsT=xewT_sb[:, k, m * 128:(m + 1) * 128],
                                             rhs=xt_sb[:, k, :],
                                             start=(k == 0), stop=(k == 3))
                        for j in range(2):
                            b = 2 * p + j
                            nc.scalar.activation(
                                out=th_sb[:, m, j * T:(j + 1) * T],
                                in_=xp_ps[:, j * T:(j + 1) * T],
                                func=AF.Tanh,
                                bias=sProjB_sb[:, m, b:b + 1], scale=1.0)
                    return th_sb

                def emit_scores(p, th_sb, scores_sb, bl2):
                    """scores for both b of pair p -> rows [bl2, bl2+1]."""
                    sc_ps = SCP.tile([1, 2 * T], f32, tag="sc")
                    for c in range(4):
                        nc.tensor.matmul(sc_ps[:], lhsT=wew_sb[:, c, :],
                                         rhs=th_sb[:, c, :],
                                         start=(c == 0), stop=(c == 3))
                    sc1 = SM.tile([1, 2 * T], f32, tag="sc1")
                    nc.vector.tensor_copy(out=sc1[:], in_=sc_ps[:])
                    nc.gpsimd.dma_start(out=scores_sb[bl2:bl2 + 2, :], in_=sc1[:])

                def emit_softmax(g):
                    scores_sb = group_state[g]["scores"]
                    nmx = SM.tile([G, 1], f32, tag="nmx")
                    nc.vector.tensor_reduce(out=nmx[:], in_=scores_sb[:], axis=AX.X,
                                            op=OP.max, negate=True)
                    pr = SM.tile([G, T], f32, tag="pr")
                    sume = SM.tile([G, 1], f32, tag="sume")
                    nc.scalar.activation(out=pr[:], in_=scores_sb[:], func=AF.Exp,
                                         bias=nmx[:], scale=1.0, accum_out=sume[:])
                    rsum = SM.tile([G, 1], f32, tag="rsum")
                    nc.vector.reciprocal(out=rsum[:], in_=sume[:])
                    alpha = SM.tile([G, T], f32, tag="alpha")
                    nc.vector.tensor_scalar_mul(out=alpha[:], in0=pr[:], scalar1=rsum[:])
                    alphaT = SM.tile([128, 2, G], f16, tag="alphaT")
                    for half in range(2):
                        alT = TP.tile([128, G], f32, tag="tp")
                        nc.tensor.transpose(out=alT[:],
                                            in_=alpha[:, half * 128:(half + 1) * 128],
                                            identity=ident[:G, :G])
                        nc.vector.tensor_copy(out=alphaT[:, half, :], in_=alT[:])
                    group_state[g]["alphaT"] = alphaT

                def emit_context(g):
                    alphaT = group_state[g]["alphaT"]
                    for bl in range(G):
                        b = g * G + bl
                        xn_sb = XN.tile([128, 2, XD], f16, tag="xn_sb")
                        nc.gpsimd.dma_start(
                            out=xn_sb[:],
                            in_=xn_d[b, :, :].rearrange("(h p) x -> p h x", p=128))
                        cx_ps = CXP.tile([1, XD], f32, tag="cx")
                        for half in range(2):
                            nc.tensor.matmul(cx_ps[:],
                                             lhsT=alphaT[:, half, bl:bl + 1],
                                             rhs=xn_sb[:, half, :],
                                             start=(half == 0), stop=(half == 1))
                        cn1 = SM.tile([1, XD], f32, tag="cn1")
                        nc.vector.tensor_copy(out=cn1[:], in_=cx_ps[:])
                        nc.gpsimd.dma_start(out=ctx_nat[b:b + 1, :], in_=cn1[:])

                fcw_tiles = []
                for g in range(NG):
                    if g == 2:
                        # mid-attention prefetch: GRU weights + all fc blocks
                        nc.sync.dma_start(out=wihT_sb[:], in_=wihT_d[:, :].rearrange("(c p) n -> p c n", p=128))
                        nc.sync.dma_start(out=whhT_sb[:], in_=whhT_d[:, :].rearrange("(c p) n -> p c n", p=128))
                        nc.gpsimd.dma_start(out=bih_bc[:], in_=bih_d[0:1, :].to_broadcast([BL, 3 * SD]))
                        nc.gpsimd.dma_start(out=bhh_bc[:], in_=bhh_d[0:1, :].to_broadcast([BL, 3 * SD]))
                        nc.sync.dma_start(out=sn_sb[:], in_=sn_d[:, :])
                        for nb in range(NYB):
                            y0 = nb * YB
                            yw = min(YB, YD - y0)
                            if nb < 10:
                                fcw_t = FW.tile([128, 4, YB], f16, tag="fcw", name=f"fcw_{nb}")
                                nc.sync.dma_start(out=fcw_t[:, :, :yw], in_=fcwT_ap[:, :, y0:y0 + yw])
                                fcw_tiles.append(fcw_t)
                    scores_t = SM.tile([G, T], f32, tag="scores", name=f"scores_{g}")
                    group_state[g] = {"scores": scores_t}
                    prev_th = None
                    for pl in range(GP2):
                        p = g * GP2 + pl
                        th_sb = emit_pair(p, prev_th)
                        if pl > 0:
                            emit_scores(p - 1, prev_th, group_state[g]["scores"],
                                        2 * (pl - 1))
                        prev_th = th_sb
                        if pl == 1 and g >= 1:
                            emit_softmax(g - 1)
                        if pl == 2 and g >= 1:
                            emit_context(g - 1)
                    emit_scores(g * GP2 + GP2 - 1, prev_th,
                                group_state[g]["scores"], 2 * (GP2 - 1))
                    if g >= 1:
                        del group_state[g - 1]
                # flush last group (+ heater bridges its softmax latency)
                for hi in range(N_HEAT):
                    heat = XPP.tile([128, 2 * T], f32, tag="xp")
                    nc.tensor.matmul(heat[:], lhsT=sewT_sb[:, 0, 0:128],
                                     rhs=xewT_sb[:, 0, :], start=True, stop=True)
                emit_softmax(NG - 1)
                emit_context(NG - 1)

            # ccT chunks 4..7 = context (transpose ctx_nat)
            for c in range(4):
                cxT = TP.tile([128, BL], f32, tag="tp")
                nc.tensor.transpose(out=cxT[:], in_=ctx_nat[:, c * 128:(c + 1) * 128],
                                    identity=ident[:BL, :BL])
                nc.vector.tensor_copy(out=ccT_sb[:, 4 + c, :], in_=cxT[:])

            # ================= GRU (natural [b, 3SD] layout) ===============
            with (
                tc.tile_pool(name="g_ps", bufs=3, space="PSUM") as GP,
                tc.tile_pool(name="lg_ps", bufs=2, space="PSUM") as LGP,
                tc.tile_pool(name="lgo", bufs=3) as LO,
                tc.tile_pool(name="fcb", bufs=2) as FB,
            ):
                gib_sb = C.tile([BL, 3 * SD], f32)
                ghb_sb = C.tile([BL, 3 * SD], f32)
                for nb3 in range(3):
                    n0 = nb3 * SD
                    gi_ps = GP.tile([BL, SD], f32, tag="g")
                    for k in range(8):
                        nc.tensor.matmul(gi_ps[:], lhsT=ccT_sb[:, k, :],
                                         rhs=wihT_sb[:, k, n0:n0 + SD],
                                         start=(k == 0), stop=(k == 7))
                    nc.vector.tensor_add(out=gib_sb[:, n0:n0 + SD], in0=gi_ps[:],
                                         in1=bih_bc[:, n0:n0 + SD])
                    gh_ps = GP.tile([BL, SD], f32, tag="g")
                    for k in range(4):
                        nc.tensor.matmul(gh_ps[:], lhsT=sT16[:, k, :],
                                         rhs=whhT_sb[:, k, n0:n0 + SD],
                                         start=(k == 0), stop=(k == 3))
                    nc.vector.tensor_add(out=ghb_sb[:, n0:n0 + SD], in0=gh_ps[:],
                                         in1=bhh_bc[:, n0:n0 + SD])
                for hi in range(14):
                    heat2 = GP.tile([BL, SD], f32, tag="g", name=f"heat2_{hi}")
                    nc.tensor.matmul(heat2[:], lhsT=sT16[:, 0, :],
                                     rhs=whhT_sb[:, 0, 0:SD], start=True, stop=True)
                rz_in = C.tile([BL, 2 * SD], f32)
                nc.vector.tensor_add(out=rz_in[:], in0=gib_sb[:, 0:2 * SD],
                                     in1=ghb_sb[:, 0:2 * SD])
                rz = C.tile([BL, 2 * SD], f32)
                nc.scalar.activation(out=rz[:], in_=rz_in[:], func=AF.Sigmoid)
                ntmp = C.tile([BL, SD], f32)
                nc.vector.tensor_mul(out=ntmp[:], in0=rz[:, 0:SD], in1=ghb_sb[:, 2 * SD:])
                nc.vector.tensor_add(out=ntmp[:], in0=gib_sb[:, 2 * SD:], in1=ntmp[:])
                n_sb = C.tile([BL, SD], f32)
                nc.scalar.activation(out=n_sb[:], in_=ntmp[:], func=AF.Tanh)
                h_sb = C.tile([BL, SD], f32)
                nc.vector.tensor_sub(out=h_sb[:], in0=sn_sb[:], in1=n_sb[:])
                nc.vector.tensor_mul(out=h_sb[:], in0=rz[:, SD:], in1=h_sb[:])
                nc.vector.tensor_add(out=h_sb[:], in0=n_sb[:], in1=h_sb[:])
                nc.sync.dma_start(out=h_d[:, :], in_=h_sb[:])

                hT16 = C.tile([128, 4, BL], f16)
                for c in range(4):
                    hp = TP.tile([128, BL], f32, tag="tp")
                    nc.tensor.transpose(out=hp[:], in_=h_sb[:, c * 128:(c + 1) * 128],
                                        identity=ident[:BL, :BL])
                    nc.vector.tensor_copy(out=hT16[:, c, :], in_=hp[:])

                for nb in range(NYB):
                    y0 = nb * YB
                    yw = min(YB, YD - y0)
                    if nb < len(fcw_tiles):
                        fcw_sb = fcw_tiles[nb]
                    else:
                        fcw_sb = FW.tile([128, 4, YB], f16, tag="fcw", name=f"fcw_{nb}")
                        nc.sync.dma_start(out=fcw_sb[:, :, :yw], in_=fcwT_ap[:, :, y0:y0 + yw])
                    fcb_sb = FB.tile([BL, YB], f32)
                    nc.gpsimd.dma_start(out=fcb_sb[:, :yw],
                                        in_=fcb_d[0:1, y0:y0 + yw].to_broadcast([BL, yw]))
                    lg_ps = LGP.tile([BL, YB], f32)
                    for k in range(4):
                        nc.tensor.matmul(lg_ps[:, :yw], lhsT=hT16[:, k, :],
                                         rhs=fcw_sb[:, k, :yw],
                                         start=(k == 0), stop=(k == 3))
                    lgo_sb = LO.tile([BL, YB], f32)
                    nc.vector.tensor_add(out=lgo_sb[:, :yw], in0=lg_ps[:, :yw],
                                         in1=fcb_sb[:, :yw])
                    nc.sync.dma_start(out=logits_d[:, y0:y0 + yw], in_=lgo_sb[:, :yw])

    nc.compile()
    return nc


@functools.lru_cache(maxsize=1)
def _program():
    return _build()


def _prep_inputs(inputs):
    x = np.asarray(inputs["x"], np.float32)
    s = np.asarray(inputs["sPrev"], np.float32)[0]
    y = np.asarray(inputs["yPrev"]).astype(np.int32)

    x16 = x.astype(np.float16)
    x5 = x16.reshape(NCORES, BL // 2, 2, T, XD)
    xT = np.ascontiguousarray(x5.transpose(0, 1, 4, 2, 3)).reshape(
        NCORES, BL // 2, XD, 2 * T)
    xn = x16.reshape(NCORES, BL, T, XD)
    sn = s.reshape(NCORES, BL, SD)
    sT = np.ascontiguousarray(sn.transpose(0, 2, 1))
    yidx = y.reshape(NCORES, BL, 1)

    xe_w = np.asarray(inputs["xe_w"], np.float32)
    se_w = np.asarray(inputs["se_w"], np.float32)
    we_w = np.asarray(inputs["we_w"], np.float32)
    xsb = (np.asarray(inputs["xe_b"], np.float32)
           + np.asarray(inputs["se_b"], np.float32)).reshape(AD, 1)
    emb = np.asarray(inputs["emb"], np.float32)
    wih = np.asarray(inputs["gru_w_ih"], np.float32)
    whh = np.asarray(inputs["gru_w_hh"], np.float32)
    bih = np.asarray(inputs["gru_b_ih"], np.float32).reshape(1, 3 * SD)
    bhh = np.asarray(inputs["gru_b_hh"], np.float32).reshape(1, 3 * SD)
    fcw = np.asarray(inputs["fc_w"], np.float32)
    fcb = np.asarray(inputs["fc_b"], np.float32).reshape(1, YD)

    shared = {
        "ident": np.eye(128, dtype=np.float32),
        "xewT": np.ascontiguousarray(xe_w.T).astype(np.float16),
        "sewT": np.ascontiguousarray(se_w.T).astype(np.float16),
        "wewT": np.ascontiguousarray(we_w.reshape(1, AD).T).astype(np.float16),
        "xsb": xsb,
        "emb": emb,
        "wihT": np.ascontiguousarray(wih.T).astype(np.float16),
        "whhT": np.ascontiguousarray(whh.T).astype(np.float16),
        "bih": bih,
        "bhh": bhh,
        "fcwT": np.ascontiguousarray(fcw.T).astype(np.float16),
        "fcb": fcb,
    }
    in_maps = []
    for c in range(NCORES):
        m = dict(shared)
        m["xT"] = xT[c]
        m["xn"] = np.ascontiguousarray(xn[c])
        m["sT"] = sT[c]
        m["sn"] = np.ascontiguousarray(sn[c])
        m["yidx"] = yidx[c]
        in_maps.append(m)
    return in_maps


def kernel(**inputs):
    nc = _program()
    in_maps = _prep_inputs(inputs)
    res = run_bass_kernel_spmd(nc, in_maps, core_ids=list(range(NCORES)))
    logits = np.concatenate([r["logits"] for r in res.results], axis=0)
    h = np.concatenate([r["h_out"] for r in res.results], axis=0)
    return logits, h


# revision 12
# speedup vs baseline: 1.1282x; 1.1282x over previous
"""Trainium2 Bass kernel for nn_DecoderUnit: additive attention + GRUCell +
vocab projection, data-parallel over batch B=256 across 8 NeuronCores.

Per core (B_LOC=32), fp16 matmuls / fp32 accumulate:
  xProj: b-pairs, rhs N=512, lhsT=xe_w.T chunks            (PE)
  tanh(xProj + sProj + xe_b + se_b): per-partition bias     (ACT)
  scores: M=1 matmuls (lhsT=we_w [128,1]) -> [1,512]/pair   (PE -> DVE -> DMA)
  softmax per 8-batch group, software-pipelined emission    (DVE/ACT)
  context: M=1 matmuls (lhsT=alphaT col, rhs=x natural)     (PE -> DVE -> DMA)
  GRU in natural [b, 3SD] layout, biases broadcast          (PE/DVE/ACT)
  logits = h @ fc_w.T + fc_b, fc_w.T streamed               (PE)
"""
import functools
import numpy as np

import concourse.bass as bass
import concourse.mybir as mybir
import concourse.tile as tile
from concourse import bacc
from concourse.bass_utils import run_bass_kernel_spmd
from concourse.masks import make_identity

B, T, XD, SD, AD, YD = 256, 256, 512, 512, 512, 6625
NCORES = 8
BL = B // NCORES          # 32
G = 8                     # softmax group size
NG = BL // G              # 4 groups
GP2 = G // 2              # 4 pairs per group
NYB = 13
YB = 512
N_HEAT = 24

f32 = mybir.dt.float32
f16 = mybir.dt.float16
i32 = mybir.dt.int32
OP = mybir.AluOpType
AF = mybir.ActivationFunctionType
AX = mybir.AxisListType


def _build():
    nc = bacc.Bacc(name="decoder_unit")

    xT_d = nc.dram_tensor("xT", [BL // 2, XD, 2 * T], f16, kind="ExternalInput")
    xn_d = nc.dram_tensor("xn", [BL, T, XD], f16, kind="ExternalInput")
    sT_d = nc.dram_tensor("sT", [SD, BL], f32, kind="ExternalInput")
    sn_d = nc.dram_tensor("sn", [BL, SD], f32, kind="ExternalInput")
    yidx_d = nc.dram_tensor("yidx", [BL, 1], i32, kind="ExternalInput")
    ident_d = nc.dram_tensor("ident", [128, 128], f32, kind="ExternalInput")
    xewT_d = nc.dram_tensor("xewT", [XD, AD], f16, kind="ExternalInput")
    sewT_d = nc.dram_tensor("sewT", [SD, AD], f16, kind="ExternalInput")
    wewT_d = nc.dram_tensor("wewT", [AD, 1], f16, kind="ExternalInput")
    xsb_d = nc.dram_tensor("xsb", [AD, 1], f32, kind="ExternalInput")
    emb_d = nc.dram_tensor("emb", [YD + 1, AD], f32, kind="ExternalInput")
    wihT_d = nc.dram_tensor("wihT", [AD + XD, 3 * SD], f16, kind="ExternalInput")
    whhT_d = nc.dram_tensor("whhT", [SD, 3 * SD], f16, kind="ExternalInput")
    bih_d = nc.dram_tensor("bih", [1, 3 * SD], f32, kind="ExternalInput")
    bhh_d = nc.dram_tensor("bhh", [1, 3 * SD], f32, kind="ExternalInput")
    fcwT_d = nc.dram_tensor("fcwT", [SD, YD], f16, kind="ExternalInput")
    fcb_d = nc.dram_tensor("fcb", [1, YD], f32, kind="ExternalInput")
    logits_d = nc.dram_tensor("logits", [BL, YD], f32, kind="ExternalOutput")
    h_d = nc.dram_tensor("h_out", [BL, SD], f32, kind="ExternalOutput")

    fcwT_ap = fcwT_d[:, :].rearrange("(c p) y -> p c y", p=128)

    with tile.TileContext(nc) as tc:
        with (
            tc.tile_pool(name="const", bufs=1) as C,
            tc.tile_pool(name="sm", bufs=3) as SM,
            tc.tile_pool(name="xt", bufs=3) as XT,
            tc.tile_pool(name="xn", bufs=4) as XN,
            tc.tile_pool(name="th", bufs=3) as TH,
            tc.tile_pool(name="fcw", bufs=8) as FW,
            tc.tile_pool(name="tp_ps", bufs=1, space="PSUM") as TP,
        ):
            # --- earliest DMAs: xe_w.T + first x pair tiles ---
            ident = C.tile([128, 128], f32)
            nc.sync.dma_start(out=ident[:], in_=ident_d[:, :])
            xewT_sb = C.tile([128, 4, AD], f16)
            nc.sync.dma_start(out=xewT_sb[:], in_=xewT_d[:, :].rearrange("(c p) a -> p c a", p=128))
            sT_sb = C.tile([128, 4, BL], f32)
            nc.sync.dma_start(out=sT_sb[:], in_=sT_d[:, :].rearrange("(c p) b -> p c b", p=128))
            sewT_sb = C.tile([128, 4, AD], f16)
            nc.sync.dma_start(out=sewT_sb[:], in_=sewT_d[:, :].rearrange("(c p) a -> p c a", p=128))
            xt_tiles = {}
            for p in range(2):
                t_ = XT.tile([128, 4, 2 * T], f16, tag="xt_sb")
                nc.sync.dma_start(out=t_[:], in_=xT_d[p, :, :].rearrange("(c p2) t -> p2 c t", p2=128))
                xt_tiles[p] = t_

            # --- setup ---
            sT16 = C.tile([128, 4, BL], f16)
            nc.vector.tensor_copy(out=sT16[:], in_=sT_sb[:])
            wew_sb = C.tile([128, 4, 1], f16)
            nc.sync.dma_start(out=wew_sb[:], in_=wewT_d[:, :].rearrange("(c p) o -> p c o", p=128))
            xsb_sb = C.tile([128, 4, 1], f32)
            nc.sync.dma_start(out=xsb_sb[:], in_=xsb_d[:, :].rearrange("(c p) o -> p c o", p=128))

            warm = TP.tile([128, 128], f32, tag="tp")
            nc.tensor.transpose(out=warm[:], in_=ident[:], identity=ident[:])

            sProjB_sb = C.tile([128, 4, BL], f32)
            with tc.tile_pool(name="sproj_ps", bufs=1, space="PSUM") as SPP:
                sp_ps = SPP.tile([128, 4, BL], f32)
                for m in range(4):
                    for k in range(4):
                        nc.tensor.matmul(sp_ps[:, m, :],
                                         lhsT=sewT_sb[:, k, m * 128:(m + 1) * 128],
                                         rhs=sT16[:, k, :],
                                         start=(k == 0), stop=(k == 3))
                for m in range(4):
                    nc.vector.tensor_scalar(out=sProjB_sb[:, m, :], in0=sp_ps[:, m, :],
                                            scalar1=xsb_sb[:, m, :], scalar2=None,
                                            op0=OP.add)

            idx_sb = C.tile([BL, 1], i32)
            nc.sync.dma_start(out=idx_sb[:], in_=yidx_d[:, :])
            yproj_sb = C.tile([BL, AD], f32)
            nc.gpsimd.indirect_dma_start(
                out=yproj_sb[:], out_offset=None, in_=emb_d[:, :],
                in_offset=bass.IndirectOffsetOnAxis(ap=idx_sb[:, :1], axis=0))
            ccT_sb = C.tile([128, 8, BL], f16)
            ctx_nat = C.tile([BL, XD], f32)
            wihT_sb = C.tile([128, 8, 3 * SD], f16)
            whhT_sb = C.tile([128, 4, 3 * SD], f16)
            bih_bc = C.tile([BL, 3 * SD], f32)
            bhh_bc = C.tile([BL, 3 * SD], f32)
            sn_sb = C.tile([BL, SD], f32)
            fcb_bc = C.tile([BL, YD], f32)

            # ================= attention: software-pipelined groups ========
            with (
                tc.tile_pool(name="xp_ps", bufs=4, space="PSUM") as XPP,
                tc.tile_pool(name="sc_ps", bufs=1, space="PSUM") as SCP,
                tc.tile_pool(name="cx_ps", bufs=2, space="PSUM") as CXP,
            ):
                group_state = {}

                def emit_pair(p, th_prev, pl_ctx=None, g_ctx=None):
                    """xProj + tanh for pair p; scores for previous pair."""
                    if p in xt_tiles:
                        xt_sb = xt_tiles.pop(p)
                    else:
                        xt_sb = XT.tile([128, 4, 2 * T], f16, tag="xt_sb")
                        nc.sync.dma_start(
                            out=xt_sb[:],
                            in_=xT_d[p, :, :].rearrange("(c p2) t -> p2 c t", p2=128))
                    th_sb = TH.tile([128, 4, 2 * T], f16, tag="th_sb")
                    for m in range(4):
                        xp_ps = XPP.tile([128, 2 * T], f32, tag="xp")
                        for k in range(4):
                            nc.tensor.matmul(xp_ps[:],
                                             lhsT=xewT_sb[:, k, m * 128:(m + 1) * 128],
                                             rhs=xt_sb[:, k, :],
                                             start=(k == 0), stop=(k == 3))
                        for j in range(2):
                            b = 2 * p + j
                            nc.scalar.activation(
                                out=th_sb[:, m, j * T:(j + 1) * T],
                                in_=xp_ps[:, j * T:(j + 1) * T],
                                func=AF.Tanh,
                                bias=sProjB_sb[:, m, b:b + 1], scale=1.0)
                    return th_sb

                def emit_scores(p, th_sb, scores_sb, bl2):
                    """scores for both b of pair p -> rows [bl2, bl2+1]."""
                    sc_ps = SCP.tile([1, 2 * T], f32, tag="sc")
                    for c in range(4):
                        nc.tensor.matmul(sc_ps[:], lhsT=wew_sb[:, c, :],
                                         rhs=th_sb[:, c, :],
                                         start=(c == 0), stop=(c == 3))
                    sc1 = SM.tile([1, 2 * T], f32, tag="sc1")
                    nc.vector.tensor_copy(out=sc1[:], in_=sc_ps[:])
                    nc.gpsimd.dma_start(out=scores_sb[bl2:bl2 + 2, :], in_=sc1[:])

                def emit_softmax(g):
                    scores_sb = group_state[g]["scores"]
                    nmx = SM.tile([G, 1], f32, tag="nmx")
                    nc.vector.tensor_reduce(out=nmx[:], in_=scores_sb[:], axis=AX.X,
                                            op=OP.max, negate=True)
                    pr = SM.tile([G, T], f32, tag="pr")
                    sume = SM.tile([G, 1], f32, tag="sume")
                    nc.scalar.activation(out=pr[:], in_=scores_sb[:], func=AF.Exp,
                                         bias=nmx[:], scale=1.0, accum_out=sume[:])
                    rsum = SM.tile([G, 1], f32, tag="rsum")
                    nc.vector.reciprocal(out=rsum[:], in_=sume[:])
                    alpha = SM.tile([G, T], f32, tag="alpha")
                    nc.vector.tensor_scalar_mul(out=alpha[:], in0=pr[:], scalar1=rsum[:])
                    alphaT = SM.tile([128, 2, G], f16, tag="alphaT")
                    for half in range(2):
                        alT = TP.tile([128, G], f32, tag="tp")
                        nc.tensor.transpose(out=alT[:],
                                            in_=alpha[:, half * 128:(half + 1) * 128],
                                            identity=ident[:G, :G])
                        nc.vector.tensor_copy(out=alphaT[:, half, :], in_=alT[:])
                    group_state[g]["alphaT"] = alphaT

                def emit_context(g):
                    alphaT = group_state[g]["alphaT"]
                    for bl in range(G):
                        b = g * G + bl
                        xn_sb = XN.tile([128, 2, XD], f16, tag="xn_sb")
                        nc.sync.dma_start(
                            out=xn_sb[:],
                            in_=xn_d[b, :, :].rearrange("(h p) x -> p h x", p=128))
                        cx_ps = CXP.tile([1, XD], f32, tag="cx")
                        for half in range(2):
                            nc.tensor.matmul(cx_ps[:],
                                             lhsT=alphaT[:, half, bl:bl + 1],
                                             rhs=xn_sb[:, half, :],
                                             start=(half == 0), stop=(half == 1))
                        cn1 = SM.tile([1, XD], f32, tag="cn1")
                        nc.vector.tensor_copy(out=cn1[:], in_=cx_ps[:])
                        nc.gpsimd.dma_start(out=ctx_nat[b:b + 1, :], in_=cn1[:])

                fcw_tiles = []
                for g in range(NG):
                    if g == 1:
                        for c in range(4):
                            ydT = TP.tile([128, BL], f32, tag="tp")
                            nc.tensor.transpose(out=ydT[:], in_=yproj_sb[:, c * 128:(c + 1) * 128],
                                                identity=ident[:BL, :BL])
                            nc.vector.tensor_copy(out=ccT_sb[:, c, :], in_=ydT[:])
                        nc.gpsimd.dma_start(out=fcb_bc[:], in_=fcb_d[0:1, :].to_broadcast([BL, YD]))
                    if g == 2:
                        # mid-attention prefetch: GRU weights + all fc blocks
                        nc.sync.dma_start(out=wihT_sb[:], in_=wihT_d[:, :].rearrange("(c p) n -> p c n", p=128))
                        nc.sync.dma_start(out=whhT_sb[:], in_=whhT_d[:, :].rearrange("(c p) n -> p c n", p=128))
                        nc.gpsimd.dma_start(out=bih_bc[:], in_=bih_d[0:1, :].to_broadcast([BL, 3 * SD]))
                        nc.gpsimd.dma_start(out=bhh_bc[:], in_=bhh_d[0:1, :].to_broadcast([BL, 3 * SD]))
                        nc.sync.dma_start(out=sn_sb[:], in_=sn_d[:, :])
                        for nb in range(NYB):
                            y0 = nb * YB
                            yw = min(YB, YD - y0)
                            if nb < 8:
                                fcw_t = FW.tile([128, 4, YB], f16, tag="fcw", name=f"fcw_{nb}")
                                nc.sync.dma_start(out=fcw_t[:, :, :yw], in_=fcwT_ap[:, :, y0:y0 + yw])
                                fcw_tiles.append(fcw_t)
                    scores_t = SM.tile([G, T], f32, tag="scores", name=f"scores_{g}")
                    group_state[g] = {"scores": scores_t}
                    prev_th = None
                    for pl in range(GP2):
                        p = g * GP2 + pl
                        th_sb = emit_pair(p, prev_th)
                        if pl > 0:
                            emit_scores(p - 1, prev_th, group_state[g]["scores"],
                                        2 * (pl - 1))
                        prev_th = th_sb
                        if pl == 1 and g >= 1:
                            emit_softmax(g - 1)
                        if pl == 2 and g >= 1:
                            emit_context(g - 1)
                    emit_scores(g * GP2 + GP2 - 1, prev_th,
                                group_state[g]["scores"], 2 * (GP2 - 1))
                    if g >= 1:
                        del group_state[g - 1]
                # flush last group (+ heater bridges its softmax latency)
                for hi in range(N_HEAT):
                    heat = XPP.tile([128, 2 * T], f32, tag="xp")
                    nc.tensor.matmul(heat[:], lhsT=sewT_sb[:, 0, 0:128],
                                     rhs=xewT_sb[:, 0, :], start=True, stop=True)
                emit_softmax(NG - 1)
                emit_context(NG - 1)

            # ccT chunks 4..7 = context (transpose ctx_nat)
            for c in range(4):
                cxT = TP.tile([128, BL], f32, tag="tp")
                nc.tensor.transpose(out=cxT[:], in_=ctx_nat[:, c * 128:(c + 1) * 128],
                                    identity=ident[:BL, :BL])
                nc.vector.tensor_copy(out=ccT_sb[:, 4 + c, :], in_=cxT[:])

            # ================= GRU (natural [b, 3SD] layout) ===============
            with (
                tc.tile_pool(name="g_ps", bufs=3, space="PSUM") as GP,
                tc.tile_pool(name="lg_ps", bufs=3, space="PSUM") as LGP,
                tc.tile_pool(name="lgo", bufs=2) as LO,
            ):
                gib_sb = C.tile([BL, 3 * SD], f32)
                ghb_sb = C.tile([BL, 3 * SD], f32)
                for nb3 in range(3):
                    n0 = nb3 * SD
                    gi_ps = GP.tile([BL, SD], f32, tag="g")
                    for k in range(8):
                        nc.tensor.matmul(gi_ps[:], lhsT=ccT_sb[:, k, :],
                                         rhs=wihT_sb[:, k, n0:n0 + SD],
                                         start=(k == 0), stop=(k == 7))
                    nc.vector.tensor_add(out=gib_sb[:, n0:n0 + SD], in0=gi_ps[:],
                                         in1=bih_bc[:, n0:n0 + SD])
                    gh_ps = GP.tile([BL, SD], f32, tag="g")
                    for k in range(4):
                        nc.tensor.matmul(gh_ps[:], lhsT=sT16[:, k, :],
                                         rhs=whhT_sb[:, k, n0:n0 + SD],
                                         start=(k == 0), stop=(k == 3))
                    nc.vector.tensor_add(out=ghb_sb[:, n0:n0 + SD], in0=gh_ps[:],
                                         in1=bhh_bc[:, n0:n0 + SD])
                for hi in range(14):
                    heat2 = GP.tile([BL, SD], f32, tag="g", name=f"heat2_{hi}")
                    nc.tensor.matmul(heat2[:], lhsT=sT16[:, 0, :],
                                     rhs=whhT_sb[:, 0, 0:SD], start=True, stop=True)
                rz_in = C.tile([BL, 2 * SD], f32)
                nc.vector.tensor_add(out=rz_in[:], in0=gib_sb[:, 0:2 * SD],
                                     in1=ghb_sb[:, 0:2 * SD])
                rz = C.tile([BL, 2 * SD], f32)
                nc.scalar.activation(out=rz[:], in_=rz_in[:], func=AF.Sigmoid)
                ntmp = C.tile([BL, SD], f32)
                nc.vector.tensor_mul(out=ntmp[:], in0=rz[:, 0:SD], in1=ghb_sb[:, 2 * SD:])
                nc.vector.tensor_add(out=ntmp[:], in0=gib_sb[:, 2 * SD:], in1=ntmp[:])
                n_sb = C.tile([BL, SD], f32)
                nc.scalar.activation(out=n_sb[:], in_=ntmp[:], func=AF.Tanh)
                h_sb = C.tile([BL, SD], f32)
                nc.vector.tensor_sub(out=h_sb[:], in0=sn_sb[:], in1=n_sb[:])
                nc.vector.tensor_mul(out=h_sb[:], in0=rz[:, SD:], in1=h_sb[:])
                nc.vector.tensor_add(out=h_sb[:], in0=n_sb[:], in1=h_sb[:])
                nc.sync.dma_start(out=h_d[:, :], in_=h_sb[:])

                hT16 = C.tile([128, 4, BL], f16)
                for c in range(4):
                    hp = TP.tile([128, BL], f32, tag="tp")
                    nc.tensor.transpose(out=hp[:], in_=h_sb[:, c * 128:(c + 1) * 128],
                                        identity=ident[:BL, :BL])
                    nc.vector.tensor_copy(out=hT16[:, c, :], in_=hp[:])

                for nb in range(NYB):
                    y0 = nb * YB
                    yw = min(YB, YD - y0)
                    if nb < len(fcw_tiles):
                        fcw_sb = fcw_tiles[nb]
                    else:
                        fcw_sb = FW.tile([128, 4, YB], f16, tag="fcw", name=f"fcw_{nb}")
                        nc.sync.dma_start(out=fcw_sb[:, :, :yw], in_=fcwT_ap[:, :, y0:y0 + yw])
                    lg_ps = LGP.tile([BL, YB], f32)
                    for k in range(4):
                        nc.tensor.matmul(lg_ps[:, :yw], lhsT=hT16[:, k, :],
                                         rhs=fcw_sb[:, k, :yw],
                                         start=(k == 0), stop=(k == 3))
                    lgo_sb = LO.tile([BL, YB], f32)
                    nc.vector.tensor_add(out=lgo_sb[:, :yw], in0=lg_ps[:, :yw],
                                         in1=fcb_bc[:, y0:y0 + yw])
                    nc.sync.dma_start(out=logits_d[:, y0:y0 + yw], in_=lgo_sb[:, :yw])

    nc.compile()
    return nc


@functools.lru_cache(maxsize=1)
def _program():
    return _build()


def _prep_inputs(inputs):
    x = np.asarray(inputs["x"], np.float32)
    s = np.asarray(inputs["sPrev"], np.float32)[0]
    y = np.asarray(inputs["yPrev"]).astype(np.int32)

    x16 = x.astype(np.float16)
    x5 = x16.reshape(NCORES, BL // 2, 2, T, XD)
    xT = np.ascontiguousarray(x5.transpose(0, 1, 4, 2, 3)).reshape(
        NCORES, BL // 2, XD, 2 * T)
    xn = x16.reshape(NCORES, BL, T, XD)
    sn = s.reshape(NCORES, BL, SD)
    sT = np.ascontiguousarray(sn.transpose(0, 2, 1))
    yidx = y.reshape(NCORES, BL, 1)

    xe_w = np.asarray(inputs["xe_w"], np.float32)
    se_w = np.asarray(inputs["se_w"], np.float32)
    we_w = np.asarray(inputs["we_w"], np.float32)
    xsb = (np.asarray(inputs["xe_b"], np.float32)
           + np.asarray(inputs["se_b"], np.float32)).reshape(AD, 1)
    emb = np.asarray(inputs["emb"], np.float32)
    wih = np.asarray(inputs["gru_w_ih"], np.float32)
    whh = np.asarray(inputs["gru_w_hh"], np.float32)
    bih = np.asarray(inputs["gru_b_ih"], np.float32).reshape(1, 3 * SD)
    bhh = np.asarray(inputs["gru_b_hh"], np.float32).reshape(1, 3 * SD)
    fcw = np.asarray(inputs["fc_w"], np.float32)
    fcb = np.asarray(inputs["fc_b"], np.float32).reshape(1, YD)

    shared = {
        "ident": np.eye(128, dtype=np.float32),
        "xewT": np.ascontiguousarray(xe_w.T).astype(np.float16),
        "sewT": np.ascontiguousarray(se_w.T).astype(np.float16),
        "wewT": np.ascontiguousarray(we_w.reshape(1, AD).T).astype(np.float16),
        "xsb": xsb,
        "emb": emb,
        "wihT": np.ascontiguousarray(wih.T).astype(np.float16),
        "whhT": np.ascontiguousarray(whh.T).astype(np.float16),
        "bih": bih,
        "bhh": bhh,
        "fcwT": np.ascontiguousarray(fcw.T).astype(np.float16),
        "fcb": fcb,
    }
    in_maps = []
    for c in range(NCORES):
        m = dict(shared)
        m["xT"] = xT[c]
        m["xn"] = np.ascontiguousarray(xn[c])
        m["sT"] = sT[c]
        m["sn"] = np.ascontiguousarray(sn[c])
        m["yidx"] = yidx[c]
        in_maps.append(m)
    return in_maps


def kernel(**inputs):
    nc = _program()
    in_maps = _prep_inputs(inputs)
    res = run_bass_kernel_spmd(nc, in_maps, core_ids=list(range(NCORES)))
    logits = np.concatenate([r["logits"] for r in res.results], axis=0)
    h = np.concatenate([r["h_out"] for r in res.results], axis=0)
    return logits, h
